# revision 10
# baseline (speedup 1.0000x reference)
"""Trainium2 Bass kernel for nn_Attn_decoder_rnn (LSTM step + dot attention + vocab proj).

Sharding (8 cores):
  - batch (64) strided-sharded: core k owns global b in {8j+k}, local slot j.
    All batch-indexed tensors on device live in "o-order": o = 8k + j
    (rank-major), so an AllGather concatenation is already o-contiguous.
    Host applies perm(i) = 8*(i%8) + i//8 (self-inverse) at ship/unshard.
  - LSTM gates: contraction (E/H) sharded 128 per core + AllReduce [64,4096],
    fp32 matmuls (exact).
  - out_W: vocab-sharded columns of out_W.T, bf16.
  - encoder_outputs: per-core [S, 8, H] slab, processed in 8 rounds of 1
    batch; each round's slab stays SBUF-resident so energies (DVE
    tensor_tensor_reduce, fp32) and context (PE matmul, float32r) read it
    from SBUF with a single HBM pass.
"""

import functools
import math

import numpy as np

NCORES = 8
B = 64
H = 1024
E = 1024
FOURH = 4 * H
NR = 8  # rounds per core
RB = 1  # local batches per round
BL = NR * RB  # local batches per core


def _bf16():
    import ml_dtypes

    return np.dtype(ml_dtypes.bfloat16)


def _perm():
    # o = 8k + j  <->  b = 8j + k ; self-inverse permutation on 0..63
    return np.array([8 * (i % 8) + i // 8 for i in range(B)], dtype=np.int64)


@functools.lru_cache(maxsize=4)
def build_program(S: int, VOCAB: int):
    import concourse.bacc as bacc
    import concourse.bass as bass
    import concourse.mybir as mybir
    import concourse.tile as tile

    dt = mybir.dt
    f32 = dt.float32
    f32r = dt.float32r
    bf16 = dt.bfloat16
    i32 = dt.int32
    AX = mybir.AxisListType
    AF = mybir.ActivationFunctionType
    ALU = mybir.AluOpType

    SC = S // 128  # s-chunks total
    VL = math.ceil(VOCAB / NCORES)  # vocab cols per core (padded on host)
    NVT = math.ceil(VL / 512)  # vocab tiles per core
    HC = H // 128  # 8
    QC = 2 * H // 128  # 16 contraction chunks for concat

    nc = bacc.Bacc("TRN2", target_bir_lowering=False, debug=False, num_devices=NCORES)

    def inp(name, shape, dtype):
        return nc.dram_tensor(name, list(shape), dtype, kind="ExternalInput")

    def outp(name, shape, dtype):
        return nc.dram_tensor(name, list(shape), dtype, kind="ExternalOutput")

    seq_idx = inp("seq_idx", [B, 1], i32)  # o-order token ids
    idx_rep = inp("idx_rep", [128, BL], i32)  # col j: my o-index (repeated)
    emb_cols = inp("emb_cols", [VOCAB, 128], f32)  # emb_W[:, e-slice]
    wihT = inp("wihT", [128, FOURH], f32)  # W_ih.T[e-slice]
    whhT = inp("whhT", [128, FOURH], f32)  # W_hh.T[e-slice]
    h0T_sl = inp("h0T_sl", [128, B], f32)  # h0_o.T[e-slice]
    bias8 = inp("bias8", [1, FOURH], f32)  # (b_ih+b_hh)/8
    c0 = inp("c0", [B, H], f32)  # o-order
    enc = inp("enc", [NR, SC, 128, RB * H], f32r)  # per-core enc slabs (raw f32 bits)
    wcT = inp("wcT", [QC, 128, H], bf16)  # concat_W.T tiled by q-chunk
    cbT = inp("cbT", [128, HC], f32)  # concat_b tiled
    owT = inp("owT", [HC, 128, VL], bf16)  # out_W.T shard tiled by h-chunk
    ob = inp("ob", [1, VL], f32)  # out_b shard
    ident = inp("ident", [128, 128], f32)  # identity for PE transpose

    logits = outp("logits", [B, VL], f32)  # o-rows, vocab shard
    attn_o = outp("attn", [BL, S], f32)  # local rows j
    h_out = outp("h_out", [B, H], f32)  # o-rows (same on all cores)
    c_out = outp("c_out", [B, H], f32)

    # internal DRAM
    gates_part = nc.dram_tensor("gates_part", [B, FOURH], f32)
    gates_red = nc.dram_tensor("gates_red", [B, FOURH], f32, addr_space="Shared")
    h_bounce = nc.dram_tensor("h_bounce", [B, H], f32)
    ctx_loc = nc.dram_tensor("ctx_loc", [BL, H], f32)
    ctx_all = nc.dram_tensor("ctx_all", [B, H], f32, addr_space="Shared")

    groups = [list(range(NCORES))]

    with tile.TileContext(nc) as tc:
        with (
            tc.tile_pool(name="const", bufs=1) as constp,
            tc.tile_pool(name="encres", bufs=1) as encp,
            tc.tile_pool(name="attn", bufs=2) as attnp,
            tc.tile_pool(name="small", bufs=2) as smallp,
            tc.tile_pool(name="proj", bufs=4) as projp,
        ):
            identity = constp.tile([128, 128], f32, tag="ident")
            nc.sync.dma_start(identity[:], ident[:])
            ones_row = constp.tile([1, B], f32, tag="ones")
            nc.vector.memset(ones_row[:], 1.0)
            idx_sb = constp.tile([B, 1], i32, tag="idx")
            nc.sync.dma_start(idx_sb[:], seq_idx[:])
            idxrep_sb = constp.tile([128, BL], i32, tag="idxrep")
            nc.sync.dma_start(idxrep_sb[:], idx_rep[:])
            hT_sb = constp.tile([128, HC * B], bf16, tag="hT")
            ctxT = constp.tile([128, HC * B], bf16, tag="ctxT")
            concT = constp.tile([128, HC * B], bf16, tag="concT")
            cb_sb = constp.tile([128, HC], f32, tag="cbT")
            nc.sync.dma_start(cb_sb[:], cbT[:])

            # ====== Phase L: LSTM gates (e-contraction shard, fp32) ======
            lstmp = tc.alloc_tile_pool(name="lstm", bufs=1)
            psp = tc.alloc_tile_pool(name="psL", bufs=2, space="PSUM")

            x_sb = lstmp.tile([B, 128], f32, tag="x")
            nc.gpsimd.indirect_dma_start(
                out=x_sb[:],
                out_offset=None,
                in_=emb_cols[:],
                in_offset=bass.IndirectOffsetOnAxis(ap=idx_sb[:, :1], axis=0),
            )
            xT_ps = psp.tile([128, B], f32, tag="xT", bufs=1)
            nc.tensor.transpose(xT_ps[:], x_sb[:], identity[:B, :B])
            xT = lstmp.tile([128, B], f32, tag="xTs")
            nc.scalar.copy(xT[:], xT_ps[:])
            h0T_sb = lstmp.tile([128, B], f32, tag="h0T")
            nc.sync.dma_start(h0T_sb[:], h0T_sl[:])

            for jb in range(FOURH // 512):
                sl = slice(512 * jb, 512 * (jb + 1))
                wi = lstmp.tile([128, 512], f32, tag="wi", bufs=2)
                nc.sync.dma_start(wi[:], wihT[:, sl])
                wh = lstmp.tile([128, 512], f32, tag="wh", bufs=2)
                nc.sync.dma_start(wh[:], whhT[:, sl])
                bs = lstmp.tile([1, 512], f32, tag="bs", bufs=2)
                nc.sync.dma_start(bs[:], bias8[:, sl])
                gps = psp.tile([B, 512], f32, tag="gps")
                nc.tensor.matmul(gps[:], xT[:], wi[:], start=True, stop=False)
                nc.tensor.matmul(gps[:], h0T_sb[:], wh[:], start=False, stop=False)
                nc.tensor.matmul(
                    gps[:], ones_row[:], bs[:], start=False, stop=True
                )
                ge = lstmp.tile([B, 512], f32, tag="ge", bufs=2)
                nc.vector.tensor_copy(ge[:], gps[:])
                nc.sync.dma_start(gates_part[:, sl], ge[:])

            nc.gpsimd.collective_compute(
                "AllReduce",
                mybir.AluOpType.add,
                replica_groups=groups,
                ins=[gates_part[:]],
                outs=[gates_red[:]],
            )

            # ====== cell (natural [64, 1024], fp32, in-place) ======
            gi = lstmp.tile([B, H], f32, tag="gi")
            nc.sync.dma_start(gi[:], gates_red[:, 0:H])
            gf = lstmp.tile([B, H], f32, tag="gf")
            nc.sync.dma_start(gf[:], gates_red[:, H : 2 * H])
            gg = lstmp.tile([B, H], f32, tag="gg")
            nc.sync.dma_start(gg[:], gates_red[:, 2 * H : 3 * H])
            go = lstmp.tile([B, H], f32, tag="go")
            nc.sync.dma_start(go[:], gates_red[:, 3 * H : 4 * H])
            c0_sb = lstmp.tile([B, H], f32, tag="c0")
            nc.sync.dma_start(c0_sb[:], c0[:])

            nc.scalar.activation(gi[:], gi[:], AF.Sigmoid)  # sig(i)
            nc.scalar.activation(gf[:], gf[:], AF.Sigmoid)  # sig(f)
            nc.scalar.activation(gg[:], gg[:], AF.Tanh)  # tanh(g)
            nc.scalar.activation(go[:], go[:], AF.Sigmoid)  # sig(o)
            nc.vector.tensor_tensor(out=gf[:], in0=gf[:], in1=c0_sb[:], op=ALU.mult)
            nc.vector.tensor_tensor(out=gi[:], in0=gi[:], in1=gg[:], op=ALU.mult)
            c_sb = lstmp.tile([B, H], f32, tag="c")
            nc.vector.tensor_tensor(out=c_sb[:], in0=gf[:], in1=gi[:], op=ALU.add)
            nc.scalar.activation(gg[:], c_sb[:], AF.Tanh)  # tanh(c)
            h_sb = lstmp.tile([B, H], f32, tag="h")
            nc.vector.tensor_tensor(out=h_sb[:], in0=go[:], in1=gg[:], op=ALU.mult)

            nc.sync.dma_start(h_out[:], h_sb[:])
            nc.sync.dma_start(c_out[:], c_sb[:])
            nc.sync.dma_start(h_bounce[:], h_sb[:])

            # hT (bf16) for the concat matmul: 8 PE transposes
            for t in range(HC):
                tps = psp.tile([128, B], f32, tag="hTps")
                nc.tensor.transpose(
                    tps[:], h_sb[:, 128 * t : 128 * (t + 1)], identity[:B, :B]
                )
                nc.scalar.copy(hT_sb[:, B * t : B * (t + 1)], tps[:])
            psp.release()
            lstmp.release()

            # ====== Phase A: attention rounds (1 local batch each) ======
            psp = tc.alloc_tile_pool(name="psA", bufs=1, space="PSUM")
            for r in range(NR):
                hrep = attnp.tile([128, H], f32, tag="hrep")
                nc.gpsimd.indirect_dma_start(
                    out=hrep[:],
                    out_offset=None,
                    in_=h_bounce[:],
                    in_offset=bass.IndirectOffsetOnAxis(
                        ap=idxrep_sb[:, r : r + 1], axis=0
                    ),
                )

                eT = attnp.tile([128, SC], f32, tag="eT")
                trash = attnp.tile([128, H], f32, tag="trash")
                enc_tiles = []
                for sc in range(SC):
                    et = encp.tile([128, H], f32r, tag=f"enc{sc}", name=f"enc{sc}")
                    nc.sync.dma_start(et[:], enc[r, sc])
                    enc_tiles.append(et)
                    nc.vector.scalar_tensor_tensor(
                        out=trash[:],
                        in0=et[:].bitcast(f32),
                        scalar=1.0,
                        in1=hrep[:],
                        op0=ALU.mult,
                        op1=ALU.mult,
                        accum_out=eT[:, sc : sc + 1],
                    )

                # energies [128 s-part, sc] -> natural [1, S] via PE T + DMA
                eps = psp.tile([SC, 128], f32, tag="eps")
                nc.tensor.transpose(eps[:], eT[:], identity[:])
                e32 = smallp.tile([SC, 128], f32, tag="e32")
                nc.scalar.copy(e32[:], eps[:])
                erow = smallp.tile([1, S], f32, tag="erow")
                nc.sync.dma_start(
                    erow[:].rearrange("b (sc p) -> b sc p", sc=SC), e32[:]
                )

                mx = smallp.tile([1, 1], f32, tag="mx")
                nc.vector.reduce_max(mx[:], erow[:], axis=AX.X)
                negmx = smallp.tile([1, 1], f32, tag="negmx")
                nc.vector.tensor_scalar_mul(negmx[:], mx[:], -1.0)
                zsum = smallp.tile([1, 1], f32, tag="zsum")
                nc.scalar.activation(
                    erow[:], erow[:], AF.Exp, bias=negmx[:, :1], accum_out=zsum[:, :1]
                )
                rz = smallp.tile([1, 1], f32, tag="rz")
                nc.vector.reciprocal(rz[:], zsum[:])
                nc.vector.tensor_scalar_mul(erow[:], erow[:], rz[:, :1])
                nc.sync.dma_start(attn_o[r : r + 1, :], erow[:])

                # attnT [128 s-part, sc] via PE transposes into one psum bank
                atps = psp.tile([128, SC], f32, tag="atps")
                for sc in range(SC):
                    nc.tensor.matmul(
                        atps[:, sc : sc + 1],
                        erow[:, 128 * sc : 128 * (sc + 1)],
                        identity[:1, :1],
                        is_transpose=True,
                        start=(sc == 0),
                        stop=(sc == SC - 1),
                    )
                attnT = smallp.tile([128, SC], f32r, tag="attnT")
                nc.scalar.copy(attnT[:], atps[:])

                # context: psum accumulate over all s-chunks (f32r full-rate)
                cps = psp.tile([1, H], f32, tag="cps", bufs=2)
                for sc in range(SC):
                    for half in range(2):
                        nc.tensor.matmul(
                            cps[:, 512 * half : 512 * (half + 1)],
                            attnT[:, sc : sc + 1],
                            enc_tiles[sc][:, 512 * half : 512 * (half + 1)],
                            start=(sc == 0),
                            stop=(sc == SC - 1),
                        )
                crow = smallp.tile([1, H], f32, tag="crow", bufs=1)
                nc.scalar.copy(crow[:], cps[:])
                nc.sync.dma_start(ctx_loc[r : r + 1, :], crow[:])
            psp.release()

            # ====== Phase C: gather context, concat, projection ======
            psp = tc.alloc_tile_pool(name="psC", bufs=2, space="PSUM")
            nc.gpsimd.collective_compute(
                "AllGather",
                mybir.AluOpType.bypass,
                replica_groups=groups,
                ins=[ctx_loc[:]],
                outs=[ctx_all[:]],
            )
            ctxg = projp.tile([B, H], f32, tag="ctxg", bufs=1)
            nc.sync.dma_start(ctxg[:], ctx_all[:])
            for t in range(HC):
                tps = psp.tile([128, B], f32, tag="ctxTps")
                nc.tensor.transpose(
                    tps[:], ctxg[:, 128 * t : 128 * (t + 1)], identity[:B, :B]
                )
                nc.scalar.copy(ctxT[:, B * t : B * (t + 1)], tps[:])

            # concat_outT [128 j-low, (jt, o)] bf16 = tanh(Wc @ [h; ctx] + cb)
            for jt in range(HC):
                pc = psp.tile([128, B], f32, tag="pc")
                for qc in range(QC):
                    w = projp.tile([128, 128], bf16, tag="wc")
                    nc.sync.dma_start(w[:], wcT[qc, :, 128 * jt : 128 * (jt + 1)])
                    rhs = (
                        hT_sb[:, B * qc : B * (qc + 1)]
                        if qc < HC
                        else ctxT[:, B * (qc - HC) : B * (qc - HC + 1)]
                    )
                    nc.tensor.matmul(
                        pc[:], w[:], rhs, start=(qc == 0), stop=(qc == QC - 1)
                    )
                nc.scalar.activation(
                    concT[:, B * jt : B * (jt + 1)],
                    pc[:],
                    AF.Tanh,
                    bias=cb_sb[:, jt : jt + 1],
                )

            for vt in range(NVT):
                nv = min(512, VL - 512 * vt)
                pp = psp.tile([B, 512], f32, tag="pp")
                for hc in range(HC):
                    wt = projp.tile([128, 512], bf16, tag="owt")
                    nc.sync.dma_start(wt[:, :nv], owT[hc, :, 512 * vt : 512 * vt + nv])
                    nc.tensor.matmul(
                        pp[:, :nv],
                        concT[:, B * hc : B * (hc + 1)],
                        wt[:, :nv],
                        start=(hc == 0),
                        stop=False,
                    )
                obt = smallp.tile([1, 512], f32, tag="obt")
                nc.sync.dma_start(obt[:, :nv], ob[:, 512 * vt : 512 * vt + nv])
                nc.tensor.matmul(
                    pp[:, :nv], ones_row[:], obt[:, :nv], start=False, stop=True
                )
                lg = projp.tile([B, 512], f32, tag="lg", bufs=2)
                nc.vector.tensor_copy(lg[:, :nv], pp[:, :nv])
                nc.sync.dma_start(logits[:, 512 * vt : 512 * vt + nv], lg[:, :nv])
            psp.release()

    nc.compile()
    return nc


# ---------------- host side ----------------


def shard_inputs(inputs, S, VOCAB):
    bf16 = _bf16()
    perm = _perm()

    input_seq = np.asarray(inputs["input_seq"]).astype(np.int32).reshape(B)
    h0 = np.asarray(inputs["last_hidden"], dtype=np.float32).reshape(B, H)
    c0_ = np.asarray(inputs["last_cell"], dtype=np.float32).reshape(B, H)
    encf = np.asarray(inputs["encoder_outputs"], dtype=np.float32)
    emb_W = np.asarray(inputs["emb_W"], dtype=np.float32)
    W_ih = np.asarray(inputs["W_ih"], dtype=np.float32)
    W_hh = np.asarray(inputs["W_hh"], dtype=np.float32)
    b_ih = np.asarray(inputs["b_ih"], dtype=np.float32)
    b_hh = np.asarray(inputs["b_hh"], dtype=np.float32)
    concat_W = np.asarray(inputs["concat_W"], dtype=np.float32)
    concat_b = np.asarray(inputs["concat_b"], dtype=np.float32)
    out_W = np.asarray(inputs["out_W"], dtype=np.float32)
    out_b = np.asarray(inputs["out_b"], dtype=np.float32)

    VL = math.ceil(VOCAB / NCORES)
    SCn = S // 128

    # o-order batch tensors (common to all cores)
    seq_o = input_seq[perm].reshape(B, 1).astype(np.int32)
    h0_o = h0[perm]
    c0_o = c0_[perm]

    bias8 = ((b_ih + b_hh) / NCORES).reshape(1, FOURH).astype(np.float32)
    wcT_b = np.ascontiguousarray(concat_W.T).reshape(2 * H // 128, 128, H).astype(bf16)
    cbT = np.ascontiguousarray(concat_b.reshape(H // 128, 128).T).astype(np.float32)
    ident = np.eye(128, dtype=np.float32)

    owTT = np.ascontiguousarray(out_W.T)  # [H, VOCAB]
    pad = NCORES * VL - VOCAB
    if pad:
        owTT = np.concatenate([owTT, np.zeros((H, pad), np.float32)], axis=1)
    ob_full = np.concatenate([out_b, np.zeros(pad, np.float32)]) if pad else out_b

    in_maps = []
    for k in range(NCORES):
        esl = slice(128 * k, 128 * (k + 1))
        my_b = np.array([8 * j + k for j in range(BL)])
        enc_k = encf[:, my_b, :]  # [S, 8, H]
        enc_k = (
            enc_k.reshape(S, NR, RB, H)
            .transpose(1, 0, 2, 3)
            .reshape(NR, SCn, 128, RB * H)
        )
        idx_rep = np.tile(np.array([8 * k + j for j in range(BL)], np.int32), (128, 1))
        in_maps.append(
            {
                "seq_idx": seq_o,
                "idx_rep": idx_rep,
                "emb_cols": np.ascontiguousarray(emb_W[:, esl]),
                "wihT": np.ascontiguousarray(W_ih[:, esl].T),
                "whhT": np.ascontiguousarray(W_hh[:, esl].T),
                "h0T_sl": np.ascontiguousarray(h0_o[:, esl].T),
                "bias8": bias8,
                "c0": c0_o,
                "enc": np.ascontiguousarray(enc_k),
                "wcT": wcT_b,
                "cbT": cbT,
                "owT": np.ascontiguousarray(owTT[:, VL * k : VL * (k + 1)])
                .reshape(H // 128, 128, VL)
                .astype(bf16),
                "ob": ob_full[VL * k : VL * (k + 1)].reshape(1, VL).astype(np.float32),
                "ident": ident,
            }
        )
    return in_maps


def unshard(results, S, VOCAB):
    perm = _perm()
    VL = math.ceil(VOCAB / NCORES)
    logits_o = np.concatenate([results[k]["logits"] for k in range(NCORES)], axis=1)
    logits = logits_o[perm][:, :VOCAB]
    h = results[0]["h_out"][perm].reshape(1, B, H)
    c = results[0]["c_out"][perm].reshape(1, B, H)
    attn = np.zeros((B, S), np.float32)
    for k in range(NCORES):
        for j in range(BL):
            attn[8 * j + k] = results[k]["attn"][j]
    return (
        logits.astype(np.float32),
        h.astype(np.float32),
        c.astype(np.float32),
        attn.reshape(B, 1, S),
    )


def run_on_hw(in_maps, S, VOCAB, trace=False):
    from concourse.bass_utils import run_bass_kernel_spmd

    nc = build_program(S, VOCAB)
    return run_bass_kernel_spmd(nc, in_maps, list(range(NCORES)), trace=trace)


def kernel(**inputs):
    encf = np.asarray(inputs["encoder_outputs"])
    S = encf.shape[0]
    VOCAB = np.asarray(inputs["emb_W"]).shape[0]
    in_maps = shard_inputs(inputs, S, VOCAB)
    res = run_on_hw(in_maps, S, VOCAB, trace=False)
    return unshard(res.results, S, VOCAB)


# revision 12
# speedup vs baseline: 1.2563x; 1.2563x over previous
"""Trainium2 Bass kernel for nn_Attn_decoder_rnn (LSTM step + dot attention + vocab proj).

Sharding (8 cores):
  - batch (64) strided-sharded: core k owns global b in {8j+k}, local slot j.
    All batch-indexed tensors on device live in "o-order": o = 8k + j
    (rank-major), so an AllGather concatenation is already o-contiguous.
    Host applies perm(i) = 8*(i%8) + i//8 (self-inverse) at ship/unshard.
  - LSTM gates: contraction (E/H) sharded 128 per core + AllReduce [64,4096],
    fp32 matmuls (exact).
  - out_W: vocab-sharded columns of out_W.T, bf16.
  - encoder_outputs: per-core [S, 8, H] slab, processed in 8 rounds of 1
    batch; each round's slab stays SBUF-resident so energies (DVE
    tensor_tensor_reduce, fp32) and context (PE matmul, float32r) read it
    from SBUF with a single HBM pass.
"""

import functools
import math

import numpy as np

NCORES = 8
B = 64
H = 1024
E = 1024
FOURH = 4 * H
NR = 8  # rounds per core
RB = 1  # local batches per round
BL = NR * RB  # local batches per core


def _bf16():
    import ml_dtypes

    return np.dtype(ml_dtypes.bfloat16)


def _perm():
    # o = 8k + j  <->  b = 8j + k ; self-inverse permutation on 0..63
    return np.array([8 * (i % 8) + i // 8 for i in range(B)], dtype=np.int64)


@functools.lru_cache(maxsize=4)
def build_program(S: int, VOCAB: int):
    import concourse.bacc as bacc
    import concourse.bass as bass
    import concourse.mybir as mybir
    import concourse.tile as tile

    dt = mybir.dt
    f32 = dt.float32
    f32r = dt.float32r
    bf16 = dt.bfloat16
    i32 = dt.int32
    AX = mybir.AxisListType
    AF = mybir.ActivationFunctionType
    ALU = mybir.AluOpType

    SC = S // 128  # s-chunks total
    VL = math.ceil(VOCAB / NCORES)  # vocab cols per core (padded on host)
    NVT = math.ceil(VL / 512)  # vocab tiles per core
    HC = H // 128  # 8
    QC = 2 * H // 128  # 16 contraction chunks for concat

    nc = bacc.Bacc("TRN2", target_bir_lowering=False, debug=False, num_devices=NCORES)

    def inp(name, shape, dtype):
        return nc.dram_tensor(name, list(shape), dtype, kind="ExternalInput")

    def outp(name, shape, dtype):
        return nc.dram_tensor(name, list(shape), dtype, kind="ExternalOutput")

    seq_idx = inp("seq_idx", [B, 1], i32)  # o-order token ids
    idx_rep = inp("idx_rep", [128, BL], i32)  # col j: my o-index (repeated)
    emb_cols = inp("emb_cols", [VOCAB, 128], f32)  # emb_W[:, e-slice]
    wihT = inp("wihT", [128, FOURH], f32)  # W_ih.T[e-slice]
    whhT = inp("whhT", [128, FOURH], f32)  # W_hh.T[e-slice]
    h0T_sl = inp("h0T_sl", [128, B], f32)  # h0_o.T[e-slice]
    bias8 = inp("bias8", [1, FOURH], f32)  # (b_ih+b_hh)/8
    c0 = inp("c0", [B, H], f32)  # o-order
    enc = inp("enc", [NR, SC, 128, RB * H], f32r)  # per-core enc slabs (raw f32 bits)
    wcT = inp("wcT", [QC, 128, H], bf16)  # concat_W.T tiled by q-chunk
    cbT = inp("cbT", [128, HC], f32)  # concat_b tiled
    owT = inp("owT", [HC, 128, VL], bf16)  # out_W.T shard tiled by h-chunk
    ob = inp("ob", [1, VL], f32)  # out_b shard
    ident = inp("ident", [128, 128], f32)  # identity for PE transpose

    logits = outp("logits", [B, VL], f32)  # o-rows, vocab shard
    attn_o = outp("attn", [BL, S], f32)  # local rows j
    h_out = outp("h_out", [B, H], f32)  # o-rows (same on all cores)
    c_out = outp("c_out", [B, H], f32)

    # internal DRAM
    gates_part = nc.dram_tensor("gates_part", [B, FOURH], f32)
    gates_red = nc.dram_tensor("gates_red", [B, FOURH], f32, addr_space="Shared")
    h_bounce = nc.dram_tensor("h_bounce", [B, H], f32)
    ctx_loc = nc.dram_tensor("ctx_loc", [BL, H], f32)
    ctx_all = nc.dram_tensor("ctx_all", [B, H], f32, addr_space="Shared")

    groups = [list(range(NCORES))]

    with tile.TileContext(nc) as tc:
        with (
            tc.tile_pool(name="const", bufs=1) as constp,
            tc.tile_pool(name="attn", bufs=2) as attnp,
            tc.tile_pool(name="small", bufs=2) as smallp,
            tc.tile_pool(name="proj", bufs=4) as projp,
        ):
            encp = tc.alloc_tile_pool(name="encres", bufs=1)
            identity = constp.tile([128, 128], f32, tag="ident")
            nc.sync.dma_start(identity[:], ident[:])
            ones_row = constp.tile([1, B], f32, tag="ones")
            nc.vector.memset(ones_row[:], 1.0)
            idx_sb = constp.tile([B, 1], i32, tag="idx")
            nc.sync.dma_start(idx_sb[:], seq_idx[:])
            idxrep_sb = constp.tile([128, BL], i32, tag="idxrep")
            nc.sync.dma_start(idxrep_sb[:], idx_rep[:])
            hT_sb = constp.tile([128, HC * B], bf16, tag="hT")
            ctxT = constp.tile([128, HC * B], bf16, tag="ctxT")
            concT = constp.tile([128, HC * B], bf16, tag="concT")
            cb_sb = constp.tile([128, HC], f32, tag="cbT")
            nc.sync.dma_start(cb_sb[:], cbT[:])

            # ====== Phase L: LSTM gates (e-contraction shard, fp32) ======
            lstmp = tc.alloc_tile_pool(name="lstm", bufs=1)
            psp = tc.alloc_tile_pool(name="psL", bufs=2, space="PSUM")

            x_sb = lstmp.tile([B, 128], f32, tag="x")
            nc.gpsimd.indirect_dma_start(
                out=x_sb[:],
                out_offset=None,
                in_=emb_cols[:],
                in_offset=bass.IndirectOffsetOnAxis(ap=idx_sb[:, :1], axis=0),
            )
            xT_ps = psp.tile([128, B], f32, tag="xT", bufs=1)
            nc.tensor.transpose(xT_ps[:], x_sb[:], identity[:B, :B])
            xT = lstmp.tile([128, B], f32, tag="xTs")
            nc.scalar.copy(xT[:], xT_ps[:])
            h0T_sb = lstmp.tile([128, B], f32, tag="h0T")
            nc.sync.dma_start(h0T_sb[:], h0T_sl[:])

            for jb in range(FOURH // 512):
                sl = slice(512 * jb, 512 * (jb + 1))
                wi = lstmp.tile([128, 512], f32, tag="wi", bufs=1)
                nc.sync.dma_start(wi[:], wihT[:, sl])
                wh = lstmp.tile([128, 512], f32, tag="wh", bufs=1)
                nc.sync.dma_start(wh[:], whhT[:, sl])
                bs = lstmp.tile([1, 512], f32, tag="bs", bufs=1)
                nc.sync.dma_start(bs[:], bias8[:, sl])
                gps = psp.tile([B, 512], f32, tag="gps")
                nc.tensor.matmul(gps[:], xT[:], wi[:], start=True, stop=False)
                nc.tensor.matmul(gps[:], h0T_sb[:], wh[:], start=False, stop=False)
                nc.tensor.matmul(
                    gps[:], ones_row[:], bs[:], start=False, stop=True
                )
                ge = lstmp.tile([B, 512], f32, tag="ge", bufs=2)
                nc.vector.tensor_copy(ge[:], gps[:])
                nc.sync.dma_start(gates_part[:, sl], ge[:])

            nc.gpsimd.collective_compute(
                "AllReduce",
                mybir.AluOpType.add,
                replica_groups=groups,
                ins=[gates_part[:]],
                outs=[gates_red[:]],
            )

            # ====== cell (natural [64, 1024], fp32, in-place) ======
            gi = lstmp.tile([B, H], f32, tag="gi")
            nc.sync.dma_start(gi[:], gates_red[:, 0:H])
            gf = lstmp.tile([B, H], f32, tag="gf")
            nc.sync.dma_start(gf[:], gates_red[:, H : 2 * H])
            gg = lstmp.tile([B, H], f32, tag="gg")
            nc.sync.dma_start(gg[:], gates_red[:, 2 * H : 3 * H])
            go = lstmp.tile([B, H], f32, tag="go")
            nc.sync.dma_start(go[:], gates_red[:, 3 * H : 4 * H])
            c0_sb = lstmp.tile([B, H], f32, tag="c0")
            nc.sync.dma_start(c0_sb[:], c0[:])

            nc.scalar.activation(gi[:], gi[:], AF.Sigmoid)  # sig(i)
            nc.scalar.activation(gf[:], gf[:], AF.Sigmoid)  # sig(f)
            nc.scalar.activation(gg[:], gg[:], AF.Tanh)  # tanh(g)
            nc.scalar.activation(go[:], go[:], AF.Sigmoid)  # sig(o)
            nc.vector.tensor_tensor(out=gf[:], in0=gf[:], in1=c0_sb[:], op=ALU.mult)
            nc.vector.tensor_tensor(out=gi[:], in0=gi[:], in1=gg[:], op=ALU.mult)
            c_sb = lstmp.tile([B, H], f32, tag="c")
            nc.vector.tensor_tensor(out=c_sb[:], in0=gf[:], in1=gi[:], op=ALU.add)
            nc.scalar.activation(gg[:], c_sb[:], AF.Tanh)  # tanh(c)
            h_sb = lstmp.tile([B, H], f32, tag="h")
            nc.vector.tensor_tensor(out=h_sb[:], in0=go[:], in1=gg[:], op=ALU.mult)

            nc.sync.dma_start(h_out[:], h_sb[:])
            nc.sync.dma_start(c_out[:], c_sb[:])
            nc.sync.dma_start(h_bounce[:], h_sb[:])

            # hT (bf16) for the concat matmul: 8 PE transposes
            for t in range(HC):
                tps = psp.tile([128, B], f32, tag="hTps")
                nc.tensor.transpose(
                    tps[:], h_sb[:, 128 * t : 128 * (t + 1)], identity[:B, :B]
                )
                nc.scalar.copy(hT_sb[:, B * t : B * (t + 1)], tps[:])
            psp.release()
            lstmp.release()

            # ====== Phase A: attention rounds (1 local batch each) ======
            psp = tc.alloc_tile_pool(name="psA", bufs=1, space="PSUM")
            for r in range(NR):
                hrep = attnp.tile([128, H], f32, tag="hrep")
                nc.gpsimd.indirect_dma_start(
                    out=hrep[:],
                    out_offset=None,
                    in_=h_bounce[:],
                    in_offset=bass.IndirectOffsetOnAxis(
                        ap=idxrep_sb[:, r : r + 1], axis=0
                    ),
                )

                eT = attnp.tile([128, SC], f32, tag="eT")
                trash = attnp.tile([128, H], f32, tag="trash", bufs=1)
                enc_tiles = []
                for sc in range(SC):
                    et = encp.tile([128, H], f32r, tag=f"enc{sc}", name=f"enc{sc}", bufs=2 if sc < 8 else 1)
                    nc.sync.dma_start(et[:], enc[r, sc])
                    enc_tiles.append(et)
                    nc.vector.scalar_tensor_tensor(
                        out=trash[:],
                        in0=et[:].bitcast(f32),
                        scalar=1.0,
                        in1=hrep[:],
                        op0=ALU.mult,
                        op1=ALU.mult,
                        accum_out=eT[:, sc : sc + 1],
                    )

                # energies [128 s-part, sc] -> natural [1, S] via PE T + DMA
                eps = psp.tile([SC, 128], f32, tag="eps")
                nc.tensor.transpose(eps[:], eT[:], identity[:])
                e32 = smallp.tile([SC, 128], f32, tag="e32")
                nc.scalar.copy(e32[:], eps[:])
                erow = smallp.tile([1, S], f32, tag="erow")
                nc.sync.dma_start(
                    erow[:].rearrange("b (sc p) -> b sc p", sc=SC), e32[:]
                )

                mx = smallp.tile([1, 1], f32, tag="mx")
                nc.vector.reduce_max(mx[:], erow[:], axis=AX.X)
                negmx = smallp.tile([1, 1], f32, tag="negmx")
                nc.vector.tensor_scalar_mul(negmx[:], mx[:], -1.0)
                zsum = smallp.tile([1, 1], f32, tag="zsum")
                nc.scalar.activation(
                    erow[:], erow[:], AF.Exp, bias=negmx[:, :1], accum_out=zsum[:, :1]
                )
                rz = smallp.tile([1, 1], f32, tag="rz")
                nc.vector.reciprocal(rz[:], zsum[:])
                nc.vector.tensor_scalar_mul(erow[:], erow[:], rz[:, :1])
                nc.sync.dma_start(attn_o[r : r + 1, :], erow[:])

                # attnT [128 s-part, sc] via PE transposes into one psum bank
                atps = psp.tile([128, SC], f32, tag="atps")
                for sc in range(SC):
                    nc.tensor.matmul(
                        atps[:, sc : sc + 1],
                        erow[:, 128 * sc : 128 * (sc + 1)],
                        identity[:1, :1],
                        is_transpose=True,
                        start=(sc == 0),
                        stop=(sc == SC - 1),
                    )
                attnT = smallp.tile([128, SC], f32r, tag="attnT")
                nc.scalar.copy(attnT[:], atps[:])

                # context: psum accumulate over all s-chunks (f32r full-rate)
                cps = psp.tile([1, H], f32, tag="cps", bufs=2)
                for sc in range(SC):
                    for half in range(2):
                        nc.tensor.matmul(
                            cps[:, 512 * half : 512 * (half + 1)],
                            attnT[:, sc : sc + 1],
                            enc_tiles[sc][:, 512 * half : 512 * (half + 1)],
                            start=(sc == 0),
                            stop=(sc == SC - 1),
                        )
                crow = smallp.tile([1, H], f32, tag="crow", bufs=1)
                nc.scalar.copy(crow[:], cps[:])
                nc.sync.dma_start(ctx_loc[r : r + 1, :], crow[:])
            psp.release()
            encp.release()

            # ====== Phase C: gather context, concat, projection ======
            psp = tc.alloc_tile_pool(name="psC", bufs=2, space="PSUM")
            projc = tc.alloc_tile_pool(name="projc", bufs=1)
            nc.gpsimd.collective_compute(
                "AllGather",
                mybir.AluOpType.bypass,
                replica_groups=groups,
                ins=[ctx_loc[:]],
                outs=[ctx_all[:]],
            )
            ctxg = projp.tile([B, H], f32, tag="ctxg", bufs=1)
            nc.sync.dma_start(ctxg[:], ctx_all[:])
            for t in range(HC):
                tps = psp.tile([128, B], f32, tag="ctxTps")
                nc.tensor.transpose(
                    tps[:], ctxg[:, 128 * t : 128 * (t + 1)], identity[:B, :B]
                )
                nc.scalar.copy(ctxT[:, B * t : B * (t + 1)], tps[:])

            # concat_outT [128 j-low, (jt, o)] bf16 = tanh(Wc @ [h; ctx] + cb)
            pc = psp.tile([128, HC * B], f32, tag="pc", bufs=1)
            for qc in range(QC):
                wq = projc.tile([128, H], bf16, tag="wq", name="wq", bufs=3)
                nc.sync.dma_start(wq[:], wcT[qc])
                rhs = (
                    hT_sb[:, B * qc : B * (qc + 1)]
                    if qc < HC
                    else ctxT[:, B * (qc - HC) : B * (qc - HC + 1)]
                )
                for jt in range(HC):
                    nc.tensor.matmul(
                        pc[:, B * jt : B * (jt + 1)],
                        wq[:, 128 * jt : 128 * (jt + 1)],
                        rhs,
                        start=(qc == 0 and jt == 0),
                        stop=(qc == QC - 1 and jt == HC - 1),
                    )
            for jt in range(HC):
                nc.scalar.activation(
                    concT[:, B * jt : B * (jt + 1)],
                    pc[:, B * jt : B * (jt + 1)],
                    AF.Tanh,
                    bias=cb_sb[:, jt : jt + 1],
                )

            owt_sb = []
            for hc in range(HC):
                wt = projc.tile([128, VL], bf16, tag=f"owt{hc}", name=f"owt{hc}")
                nc.sync.dma_start(wt[:], owT[hc])
                owt_sb.append(wt)
            for vt in range(NVT):
                nv = min(512, VL - 512 * vt)
                pp = psp.tile([B, 512], f32, tag="pp")
                for hc in range(HC):
                    nc.tensor.matmul(
                        pp[:, :nv],
                        concT[:, B * hc : B * (hc + 1)],
                        owt_sb[hc][:, 512 * vt : 512 * vt + nv],
                        start=(hc == 0),
                        stop=False,
                    )
                obt = smallp.tile([1, 512], f32, tag="obt")
                nc.sync.dma_start(obt[:, :nv], ob[:, 512 * vt : 512 * vt + nv])
                nc.tensor.matmul(
                    pp[:, :nv], ones_row[:], obt[:, :nv], start=False, stop=True
                )
                lg = projp.tile([B, 512], f32, tag="lg", bufs=2)
                nc.vector.tensor_copy(lg[:, :nv], pp[:, :nv])
                nc.sync.dma_start(logits[:, 512 * vt : 512 * vt + nv], lg[:, :nv])
            psp.release()
            projc.release()

    nc.compile()
    return nc


# ---------------- host side ----------------


def shard_inputs(inputs, S, VOCAB):
    bf16 = _bf16()
    perm = _perm()

    input_seq = np.asarray(inputs["input_seq"]).astype(np.int32).reshape(B)
    h0 = np.asarray(inputs["last_hidden"], dtype=np.float32).reshape(B, H)
    c0_ = np.asarray(inputs["last_cell"], dtype=np.float32).reshape(B, H)
    encf = np.asarray(inputs["encoder_outputs"], dtype=np.float32)
    emb_W = np.asarray(inputs["emb_W"], dtype=np.float32)
    W_ih = np.asarray(inputs["W_ih"], dtype=np.float32)
    W_hh = np.asarray(inputs["W_hh"], dtype=np.float32)
    b_ih = np.asarray(inputs["b_ih"], dtype=np.float32)
    b_hh = np.asarray(inputs["b_hh"], dtype=np.float32)
    concat_W = np.asarray(inputs["concat_W"], dtype=np.float32)
    concat_b = np.asarray(inputs["concat_b"], dtype=np.float32)
    out_W = np.asarray(inputs["out_W"], dtype=np.float32)
    out_b = np.asarray(inputs["out_b"], dtype=np.float32)

    VL = math.ceil(VOCAB / NCORES)
    SCn = S // 128

    # o-order batch tensors (common to all cores)
    seq_o = input_seq[perm].reshape(B, 1).astype(np.int32)
    h0_o = h0[perm]
    c0_o = c0_[perm]

    bias8 = ((b_ih + b_hh) / NCORES).reshape(1, FOURH).astype(np.float32)
    wcT_b = np.ascontiguousarray(concat_W.T).reshape(2 * H // 128, 128, H).astype(bf16)
    cbT = np.ascontiguousarray(concat_b.reshape(H // 128, 128).T).astype(np.float32)
    ident = np.eye(128, dtype=np.float32)

    owTT = np.ascontiguousarray(out_W.T)  # [H, VOCAB]
    pad = NCORES * VL - VOCAB
    if pad:
        owTT = np.concatenate([owTT, np.zeros((H, pad), np.float32)], axis=1)
    ob_full = np.concatenate([out_b, np.zeros(pad, np.float32)]) if pad else out_b

    in_maps = []
    for k in range(NCORES):
        esl = slice(128 * k, 128 * (k + 1))
        my_b = np.array([8 * j + k for j in range(BL)])
        enc_k = encf[:, my_b, :]  # [S, 8, H]
        enc_k = (
            enc_k.reshape(S, NR, RB, H)
            .transpose(1, 0, 2, 3)
            .reshape(NR, SCn, 128, RB * H)
        )
        idx_rep = np.tile(np.array([8 * k + j for j in range(BL)], np.int32), (128, 1))
        in_maps.append(
            {
                "seq_idx": seq_o,
                "idx_rep": idx_rep,
                "emb_cols": np.ascontiguousarray(emb_W[:, esl]),
                "wihT": np.ascontiguousarray(W_ih[:, esl].T),
                "whhT": np.ascontiguousarray(W_hh[:, esl].T),
                "h0T_sl": np.ascontiguousarray(h0_o[:, esl].T),
                "bias8": bias8,
                "c0": c0_o,
                "enc": np.ascontiguousarray(enc_k),
                "wcT": wcT_b,
                "cbT": cbT,
                "owT": np.ascontiguousarray(owTT[:, VL * k : VL * (k + 1)])
                .reshape(H // 128, 128, VL)
                .astype(bf16),
                "ob": ob_full[VL * k : VL * (k + 1)].reshape(1, VL).astype(np.float32),
                "ident": ident,
            }
        )
    return in_maps


def unshard(results, S, VOCAB):
    perm = _perm()
    VL = math.ceil(VOCAB / NCORES)
    logits_o = np.concatenate([results[k]["logits"] for k in range(NCORES)], axis=1)
    logits = logits_o[perm][:, :VOCAB]
    h = results[0]["h_out"][perm].reshape(1, B, H)
    c = results[0]["c_out"][perm].reshape(1, B, H)
    attn = np.zeros((B, S), np.float32)
    for k in range(NCORES):
        for j in range(BL):
            attn[8 * j + k] = results[k]["attn"][j]
    return (
        logits.astype(np.float32),
        h.astype(np.float32),
        c.astype(np.float32),
        attn.reshape(B, 1, S),
    )


def run_on_hw(in_maps, S, VOCAB, trace=False):
    from concourse.bass_utils import run_bass_kernel_spmd

    nc = build_program(S, VOCAB)
    return run_bass_kernel_spmd(nc, in_maps, list(range(NCORES)), trace=trace)


def kernel(**inputs):
    encf = np.asarray(inputs["encoder_outputs"])
    S = encf.shape[0]
    VOCAB = np.asarray(inputs["emb_W"]).shape[0]
    in_maps = shard_inputs(inputs, S, VOCAB)
    res = run_on_hw(in_maps, S, VOCAB, trace=False)
    return unshard(res.results, S, VOCAB)


# revision 13
# speedup vs baseline: 1.2947x; 1.0306x over previous
"""Trainium2 Bass kernel for nn_Attn_decoder_rnn (LSTM step + dot attention + vocab proj).

Sharding (8 cores):
  - batch (64) strided-sharded: core k owns global b in {8j+k}, local slot j.
    All batch-indexed tensors on device live in "o-order": o = 8k + j
    (rank-major), so an AllGather concatenation is already o-contiguous.
    Host applies perm(i) = 8*(i%8) + i//8 (self-inverse) at ship/unshard.
  - LSTM gates: contraction (E/H) sharded 128 per core + AllReduce [64,4096],
    fp32 matmuls (exact).
  - out_W: vocab-sharded columns of out_W.T, bf16.
  - encoder_outputs: per-core [S, 8, H] slab, processed in 8 rounds of 1
    batch; each round's slab stays SBUF-resident so energies (DVE
    tensor_tensor_reduce, fp32) and context (PE matmul, float32r) read it
    from SBUF with a single HBM pass.
"""

import functools
import math

import numpy as np

NCORES = 8
B = 64
H = 1024
E = 1024
FOURH = 4 * H
NR = 8  # rounds per core
RB = 1  # local batches per round
BL = NR * RB  # local batches per core


def _bf16():
    import ml_dtypes

    return np.dtype(ml_dtypes.bfloat16)


def _perm():
    # o = 8k + j  <->  b = 8j + k ; self-inverse permutation on 0..63
    return np.array([8 * (i % 8) + i // 8 for i in range(B)], dtype=np.int64)


@functools.lru_cache(maxsize=4)
def build_program(S: int, VOCAB: int):
    import concourse.bacc as bacc
    import concourse.bass as bass
    import concourse.mybir as mybir
    import concourse.tile as tile

    dt = mybir.dt
    f32 = dt.float32
    f32r = dt.float32r
    bf16 = dt.bfloat16
    i32 = dt.int32
    AX = mybir.AxisListType
    AF = mybir.ActivationFunctionType
    ALU = mybir.AluOpType

    SC = S // 128  # s-chunks total
    VL = math.ceil(VOCAB / NCORES)  # vocab cols per core (padded on host)
    NVT = math.ceil(VL / 512)  # vocab tiles per core
    HC = H // 128  # 8
    QC = 2 * H // 128  # 16 contraction chunks for concat

    nc = bacc.Bacc("TRN2", target_bir_lowering=False, debug=False, num_devices=NCORES)

    def inp(name, shape, dtype):
        return nc.dram_tensor(name, list(shape), dtype, kind="ExternalInput")

    def outp(name, shape, dtype):
        return nc.dram_tensor(name, list(shape), dtype, kind="ExternalOutput")

    seq_idx = inp("seq_idx", [B, 1], i32)  # o-order token ids
    idx_rep = inp("idx_rep", [128, BL], i32)  # col j: my o-index (repeated)
    emb_cols = inp("emb_cols", [VOCAB, 128], f32)  # emb_W[:, e-slice]
    wihT = inp("wihT", [128, FOURH], f32)  # W_ih.T[e-slice]
    whhT = inp("whhT", [128, FOURH], f32)  # W_hh.T[e-slice]
    h0T_sl = inp("h0T_sl", [128, B], f32)  # h0_o.T[e-slice]
    bias8 = inp("bias8", [1, FOURH], f32)  # (b_ih+b_hh)/8
    c0 = inp("c0", [B, H], f32)  # o-order
    enc = inp("enc", [NR, SC, 128, RB * H], f32r)  # per-core enc slabs (raw f32 bits)
    wcT = inp("wcT", [QC, 128, H], bf16)  # concat_W.T tiled by q-chunk
    cbT = inp("cbT", [128, HC], f32)  # concat_b tiled
    owT = inp("owT", [HC, 128, VL], bf16)  # out_W.T shard tiled by h-chunk
    ob = inp("ob", [1, VL], f32)  # out_b shard
    ident = inp("ident", [128, 128], f32)  # identity for PE transpose

    logits = outp("logits", [B, VL], f32)  # o-rows, vocab shard
    attn_o = outp("attn", [BL, S], f32)  # local rows j
    h_out = outp("h_out", [B, H], f32)  # o-rows (same on all cores)
    c_out = outp("c_out", [B, H], f32)

    # internal DRAM
    gates_part = nc.dram_tensor("gates_part", [B, FOURH], f32)
    gates_red = nc.dram_tensor("gates_red", [B, FOURH], f32, addr_space="Shared")
    h_bounce = nc.dram_tensor("h_bounce", [B, H], f32)
    ctx_loc = nc.dram_tensor("ctx_loc", [BL, H], f32)
    ctx_all = nc.dram_tensor("ctx_all", [B, H], f32, addr_space="Shared")

    groups = [list(range(NCORES))]

    with tile.TileContext(nc) as tc:
        with (
            tc.tile_pool(name="const", bufs=1) as constp,
            tc.tile_pool(name="attn", bufs=2) as attnp,
            tc.tile_pool(name="small", bufs=2) as smallp,
            tc.tile_pool(name="proj", bufs=4) as projp,
        ):
            encp = tc.alloc_tile_pool(name="encres", bufs=1)
            identity = constp.tile([128, 128], f32, tag="ident")
            nc.sync.dma_start(identity[:], ident[:])
            ones_row = constp.tile([1, B], f32, tag="ones")
            nc.vector.memset(ones_row[:], 1.0)
            idx_sb = constp.tile([B, 1], i32, tag="idx")
            nc.sync.dma_start(idx_sb[:], seq_idx[:])
            idxrep_sb = constp.tile([128, BL], i32, tag="idxrep")
            nc.sync.dma_start(idxrep_sb[:], idx_rep[:])
            hT_sb = constp.tile([128, HC * B], bf16, tag="hT")
            ctxT = constp.tile([128, HC * B], bf16, tag="ctxT")
            concT = constp.tile([128, HC * B], bf16, tag="concT")
            cb_sb = constp.tile([128, HC], f32, tag="cbT")
            nc.sync.dma_start(cb_sb[:], cbT[:])

            # ====== Phase L: LSTM gates (e-contraction shard, fp32) ======
            lstmp = tc.alloc_tile_pool(name="lstm", bufs=1)
            psp = tc.alloc_tile_pool(name="psL", bufs=2, space="PSUM")

            x_sb = lstmp.tile([B, 128], f32, tag="x")
            nc.gpsimd.indirect_dma_start(
                out=x_sb[:],
                out_offset=None,
                in_=emb_cols[:],
                in_offset=bass.IndirectOffsetOnAxis(ap=idx_sb[:, :1], axis=0),
            )
            xT_ps = psp.tile([128, B], f32, tag="xT", bufs=1)
            nc.tensor.transpose(xT_ps[:], x_sb[:], identity[:B, :B])
            xT = lstmp.tile([128, B], f32, tag="xTs")
            nc.scalar.copy(xT[:], xT_ps[:])
            h0T_sb = lstmp.tile([128, B], f32, tag="h0T")
            nc.sync.dma_start(h0T_sb[:], h0T_sl[:])

            for jb in range(FOURH // 512):
                sl = slice(512 * jb, 512 * (jb + 1))
                wi = lstmp.tile([128, 512], f32, tag="wi", bufs=1)
                nc.sync.dma_start(wi[:], wihT[:, sl])
                wh = lstmp.tile([128, 512], f32, tag="wh", bufs=1)
                nc.sync.dma_start(wh[:], whhT[:, sl])
                bs = lstmp.tile([1, 512], f32, tag="bs", bufs=1)
                nc.sync.dma_start(bs[:], bias8[:, sl])
                gps = psp.tile([B, 512], f32, tag="gps")
                nc.tensor.matmul(gps[:], xT[:], wi[:], start=True, stop=False)
                nc.tensor.matmul(gps[:], h0T_sb[:], wh[:], start=False, stop=False)
                nc.tensor.matmul(
                    gps[:], ones_row[:], bs[:], start=False, stop=True
                )
                ge = lstmp.tile([B, 512], f32, tag="ge", bufs=2)
                nc.vector.tensor_copy(ge[:], gps[:])
                nc.sync.dma_start(gates_part[:, sl], ge[:])

            nc.gpsimd.collective_compute(
                "AllReduce",
                mybir.AluOpType.add,
                replica_groups=groups,
                ins=[gates_part[:]],
                outs=[gates_red[:]],
            )

            # ====== cell (natural [64, 1024], fp32, fully in-place) ======
            gall = lstmp.tile([B, FOURH], f32, tag="gall")
            nc.sync.dma_start(gall[:], gates_red[:])
            c0_sb = lstmp.tile([B, H], f32, tag="c0")
            nc.sync.dma_start(c0_sb[:], c0[:])

            gi = gall[:, 0:H]
            gf = gall[:, H : 2 * H]
            gg = gall[:, 2 * H : 3 * H]
            go = gall[:, 3 * H : 4 * H]
            nc.scalar.activation(gi, gi, AF.Sigmoid)
            nc.scalar.activation(gf, gf, AF.Sigmoid)
            nc.scalar.activation(gg, gg, AF.Tanh)
            nc.scalar.activation(go, go, AF.Sigmoid)
            # c0 <- sig(f)*c0 ; gi <- sig(i)*tanh(g) ; c0 <- c (sum)
            nc.vector.tensor_tensor(out=c0_sb[:], in0=gf, in1=c0_sb[:], op=ALU.mult)
            nc.vector.tensor_tensor(out=gi, in0=gi, in1=gg, op=ALU.mult)
            nc.vector.tensor_tensor(out=c0_sb[:], in0=c0_sb[:], in1=gi, op=ALU.add)
            nc.scalar.activation(gg, c0_sb[:], AF.Tanh)  # gg <- tanh(c)
            nc.vector.tensor_tensor(out=gf, in0=go, in1=gg, op=ALU.mult)  # gf <- h

            nc.sync.dma_start(h_out[:], gf)
            nc.sync.dma_start(c_out[:], c0_sb[:])
            nc.sync.dma_start(h_bounce[:], gf)

            # hT (bf16) for the concat matmul: 8 PE transposes
            for t in range(HC):
                tps = psp.tile([128, B], f32, tag="hTps")
                nc.tensor.transpose(
                    tps[:], gall[:, H + 128 * t : H + 128 * (t + 1)], identity[:B, :B]
                )
                nc.scalar.copy(hT_sb[:, B * t : B * (t + 1)], tps[:])
            psp.release()
            lstmp.release()

            # ====== Phase A: attention rounds (1 local batch each) ======
            psp = tc.alloc_tile_pool(name="psA", bufs=1, space="PSUM")
            for r in range(NR):
                hrep = attnp.tile([128, H], f32, tag="hrep")
                nc.gpsimd.indirect_dma_start(
                    out=hrep[:],
                    out_offset=None,
                    in_=h_bounce[:],
                    in_offset=bass.IndirectOffsetOnAxis(
                        ap=idxrep_sb[:, r : r + 1], axis=0
                    ),
                )

                eT = attnp.tile([128, SC], f32, tag="eT")
                trash = attnp.tile([128, H], f32, tag="trash", bufs=1)
                enc_tiles = []
                for sc in range(SC):
                    et = encp.tile([128, H], f32r, tag=f"enc{sc}", name=f"enc{sc}", bufs=2 if sc < 14 else 1)
                    nc.sync.dma_start(et[:], enc[r, sc])
                    enc_tiles.append(et)
                    nc.vector.scalar_tensor_tensor(
                        out=trash[:],
                        in0=et[:].bitcast(f32),
                        scalar=1.0,
                        in1=hrep[:],
                        op0=ALU.mult,
                        op1=ALU.mult,
                        accum_out=eT[:, sc : sc + 1],
                    )

                # energies [128 s-part, sc] -> natural [1, S] via PE T + DMA
                eps = psp.tile([SC, 128], f32, tag="eps")
                nc.tensor.transpose(eps[:], eT[:], identity[:])
                e32 = smallp.tile([SC, 128], f32, tag="e32")
                nc.scalar.copy(e32[:], eps[:])
                erow = smallp.tile([1, S], f32, tag="erow")
                nc.sync.dma_start(
                    erow[:].rearrange("b (sc p) -> b sc p", sc=SC), e32[:]
                )

                mx = smallp.tile([1, 1], f32, tag="mx")
                nc.vector.reduce_max(mx[:], erow[:], axis=AX.X)
                negmx = smallp.tile([1, 1], f32, tag="negmx")
                nc.scalar.mul(negmx[:], mx[:], -1.0)
                zsum = smallp.tile([1, 1], f32, tag="zsum")
                nc.scalar.activation(
                    erow[:], erow[:], AF.Exp, bias=negmx[:, :1], accum_out=zsum[:, :1]
                )
                rz = smallp.tile([1, 1], f32, tag="rz")
                nc.vector.reciprocal(rz[:], zsum[:])
                nc.scalar.mul(erow[:], erow[:], rz[:, :1])
                nc.sync.dma_start(attn_o[r : r + 1, :], erow[:])

                # attnT [128 s-part, sc] via PE transposes into one psum bank
                atps = psp.tile([128, SC], f32, tag="atps")
                for sc in range(SC):
                    nc.tensor.matmul(
                        atps[:, sc : sc + 1],
                        erow[:, 128 * sc : 128 * (sc + 1)],
                        identity[:1, :1],
                        is_transpose=True,
                        start=(sc == 0),
                        stop=(sc == SC - 1),
                    )
                attnT = smallp.tile([128, SC], f32r, tag="attnT")
                nc.scalar.copy(attnT[:], atps[:])

                # context: psum accumulate over all s-chunks (f32r full-rate)
                cps = psp.tile([1, H], f32, tag="cps", bufs=2)
                for sc in range(SC):
                    for half in range(2):
                        nc.tensor.matmul(
                            cps[:, 512 * half : 512 * (half + 1)],
                            attnT[:, sc : sc + 1],
                            enc_tiles[sc][:, 512 * half : 512 * (half + 1)],
                            start=(sc == 0),
                            stop=(sc == SC - 1),
                        )
                crow = smallp.tile([1, H], f32, tag="crow", bufs=1)
                nc.scalar.copy(crow[:], cps[:])
                nc.sync.dma_start(ctx_loc[r : r + 1, :], crow[:])
            psp.release()
            encp.release()

            # ====== Phase C: gather context, concat, projection ======
            psp = tc.alloc_tile_pool(name="psC", bufs=2, space="PSUM")
            projc = tc.alloc_tile_pool(name="projc", bufs=1)
            nc.gpsimd.collective_compute(
                "AllGather",
                mybir.AluOpType.bypass,
                replica_groups=groups,
                ins=[ctx_loc[:]],
                outs=[ctx_all[:]],
            )
            ctxg = projp.tile([B, H], f32, tag="ctxg", bufs=1)
            nc.sync.dma_start(ctxg[:], ctx_all[:])
            for t in range(HC):
                tps = psp.tile([128, B], f32, tag="ctxTps")
                nc.tensor.transpose(
                    tps[:], ctxg[:, 128 * t : 128 * (t + 1)], identity[:B, :B]
                )
                nc.scalar.copy(ctxT[:, B * t : B * (t + 1)], tps[:])

            # concat_outT [128 j-low, (jt, o)] bf16 = tanh(Wc @ [h; ctx] + cb)
            pc = psp.tile([128, HC * B], f32, tag="pc", bufs=1)
            for qc in range(QC):
                wq = projc.tile([128, H], bf16, tag="wq", name="wq", bufs=3)
                nc.sync.dma_start(wq[:], wcT[qc])
                rhs = (
                    hT_sb[:, B * qc : B * (qc + 1)]
                    if qc < HC
                    else ctxT[:, B * (qc - HC) : B * (qc - HC + 1)]
                )
                for jt in range(HC):
                    nc.tensor.matmul(
                        pc[:, B * jt : B * (jt + 1)],
                        wq[:, 128 * jt : 128 * (jt + 1)],
                        rhs,
                        start=(qc == 0 and jt == 0),
                        stop=(qc == QC - 1 and jt == HC - 1),
                    )
            for jt in range(HC):
                nc.scalar.activation(
                    concT[:, B * jt : B * (jt + 1)],
                    pc[:, B * jt : B * (jt + 1)],
                    AF.Tanh,
                    bias=cb_sb[:, jt : jt + 1],
                )

            owt_sb = []
            for hc in range(HC):
                wt = projc.tile([128, VL], bf16, tag=f"owt{hc}", name=f"owt{hc}")
                nc.sync.dma_start(wt[:], owT[hc])
                owt_sb.append(wt)
            for vt in range(NVT):
                nv = min(512, VL - 512 * vt)
                pp = psp.tile([B, 512], f32, tag="pp")
                for hc in range(HC):
                    nc.tensor.matmul(
                        pp[:, :nv],
                        concT[:, B * hc : B * (hc + 1)],
                        owt_sb[hc][:, 512 * vt : 512 * vt + nv],
                        start=(hc == 0),
                        stop=False,
                    )
                obt = smallp.tile([1, 512], f32, tag="obt")
                nc.sync.dma_start(obt[:, :nv], ob[:, 512 * vt : 512 * vt + nv])
                nc.tensor.matmul(
                    pp[:, :nv], ones_row[:], obt[:, :nv], start=False, stop=True
                )
                lg = projp.tile([B, 512], f32, tag="lg", bufs=2)
                nc.vector.tensor_copy(lg[:, :nv], pp[:, :nv])
                nc.sync.dma_start(logits[:, 512 * vt : 512 * vt + nv], lg[:, :nv])
            psp.release()
            projc.release()

    nc.compile()
    return nc


# ---------------- host side ----------------


def shard_inputs(inputs, S, VOCAB):
    bf16 = _bf16()
    perm = _perm()

    input_seq = np.asarray(inputs["input_seq"]).astype(np.int32).reshape(B)
    h0 = np.asarray(inputs["last_hidden"], dtype=np.float32).reshape(B, H)
    c0_ = np.asarray(inputs["last_cell"], dtype=np.float32).reshape(B, H)
    encf = np.asarray(inputs["encoder_outputs"], dtype=np.float32)
    emb_W = np.asarray(inputs["emb_W"], dtype=np.float32)
    W_ih = np.asarray(inputs["W_ih"], dtype=np.float32)
    W_hh = np.asarray(inputs["W_hh"], dtype=np.float32)
    b_ih = np.asarray(inputs["b_ih"], dtype=np.float32)
    b_hh = np.asarray(inputs["b_hh"], dtype=np.float32)
    concat_W = np.asarray(inputs["concat_W"], dtype=np.float32)
    concat_b = np.asarray(inputs["concat_b"], dtype=np.float32)
    out_W = np.asarray(inputs["out_W"], dtype=np.float32)
    out_b = np.asarray(inputs["out_b"], dtype=np.float32)

    VL = math.ceil(VOCAB / NCORES)
    SCn = S // 128

    # o-order batch tensors (common to all cores)
    seq_o = input_seq[perm].reshape(B, 1).astype(np.int32)
    h0_o = h0[perm]
    c0_o = c0_[perm]

    bias8 = ((b_ih + b_hh) / NCORES).reshape(1, FOURH).astype(np.float32)
    wcT_b = np.ascontiguousarray(concat_W.T).reshape(2 * H // 128, 128, H).astype(bf16)
    cbT = np.ascontiguousarray(concat_b.reshape(H // 128, 128).T).astype(np.float32)
    ident = np.eye(128, dtype=np.float32)

    owTT = np.ascontiguousarray(out_W.T)  # [H, VOCAB]
    pad = NCORES * VL - VOCAB
    if pad:
        owTT = np.concatenate([owTT, np.zeros((H, pad), np.float32)], axis=1)
    ob_full = np.concatenate([out_b, np.zeros(pad, np.float32)]) if pad else out_b

    in_maps = []
    for k in range(NCORES):
        esl = slice(128 * k, 128 * (k + 1))
        my_b = np.array([8 * j + k for j in range(BL)])
        enc_k = encf[:, my_b, :]  # [S, 8, H]
        enc_k = (
            enc_k.reshape(S, NR, RB, H)
            .transpose(1, 0, 2, 3)
            .reshape(NR, SCn, 128, RB * H)
        )
        idx_rep = np.tile(np.array([8 * k + j for j in range(BL)], np.int32), (128, 1))
        in_maps.append(
            {
                "seq_idx": seq_o,
                "idx_rep": idx_rep,
                "emb_cols": np.ascontiguousarray(emb_W[:, esl]),
                "wihT": np.ascontiguousarray(W_ih[:, esl].T),
                "whhT": np.ascontiguousarray(W_hh[:, esl].T),
                "h0T_sl": np.ascontiguousarray(h0_o[:, esl].T),
                "bias8": bias8,
                "c0": c0_o,
                "enc": np.ascontiguousarray(enc_k),
                "wcT": wcT_b,
                "cbT": cbT,
                "owT": np.ascontiguousarray(owTT[:, VL * k : VL * (k + 1)])
                .reshape(H // 128, 128, VL)
                .astype(bf16),
                "ob": ob_full[VL * k : VL * (k + 1)].reshape(1, VL).astype(np.float32),
                "ident": ident,
            }
        )
    return in_maps


def unshard(results, S, VOCAB):
    perm = _perm()
    VL = math.ceil(VOCAB / NCORES)
    logits_o = np.concatenate([results[k]["logits"] for k in range(NCORES)], axis=1)
    logits = logits_o[perm][:, :VOCAB]
    h = results[0]["h_out"][perm].reshape(1, B, H)
    c = results[0]["c_out"][perm].reshape(1, B, H)
    attn = np.zeros((B, S), np.float32)
    for k in range(NCORES):
        for j in range(BL):
            attn[8 * j + k] = results[k]["attn"][j]
    return (
        logits.astype(np.float32),
        h.astype(np.float32),
        c.astype(np.float32),
        attn.reshape(B, 1, S),
    )


def run_on_hw(in_maps, S, VOCAB, trace=False):
    from concourse.bass_utils import run_bass_kernel_spmd

    nc = build_program(S, VOCAB)
    return run_bass_kernel_spmd(nc, in_maps, list(range(NCORES)), trace=trace)


def kernel(**inputs):
    encf = np.asarray(inputs["encoder_outputs"])
    S = encf.shape[0]
    VOCAB = np.asarray(inputs["emb_W"]).shape[0]
    in_maps = shard_inputs(inputs, S, VOCAB)
    res = run_on_hw(in_maps, S, VOCAB, trace=False)
    return unshard(res.results, S, VOCAB)
